# revision 27
# baseline (speedup 1.0000x reference)
"""Causal self-attention (RoPE) Trainium2 kernel.

Sharding: 2 batches x 16 heads = 32 (b,h) units over 8 cores -> each core
handles 1 batch x 4 heads. Column-parallel QKV + row-parallel output
projection; host sums the 4 partial outputs per batch (fp16 partials).

All matmul operands are fp16 (1 cycle/row on the PE; fp32 PSUM
accumulation). Host pre-casts x and the weight shards to fp16.

Per-core structure (single PE-dense stream):

  V: pass A is cc-outer over 8 PSUM accumulators and consumes only the
     first half of each x chunk, so the PE starts on the first arriving
     chunk and pass A's DMA need (6 MB) stays under the HBM rate; pass B
     computes token chunks 8..11; chunks 12..15 are deferred into the
     attention phase as PE filler for the exp-latency-bound qt0 blocks.
  QK: all 8 (q,k)-head columns, K heads first (attention needs them as
     stationary operands right after QK); [d, t] layout + RoPE on DVE.
  ATT: q-tile-outer (qt 0..3), heads inner.  S chunks are paired into
     [128, 1024] PSUM tiles (one ScalarE exp per pair, halving its fixed
     cost); a global 2-pair S/exp lookahead runs across block seams.
     proj(qt) is queued as (qc, ct) matmul groups and drained one group
     per consumed S-pair inside qt+1's blocks, so the ScalarE exp stream
     never idles and the fp16 output DMA is spread across the phase.
     The softmax denominator is accumulated on DVE (asum) + one
     ones-column matmul per block; each block's yT-normalize multiply is
     deferred into the next block so its gpsimd-broadcast wait cannot
     head-of-line-block the in-order DVE queue.

  Causal narrowing: for the 4 diagonal k-chunks of each q-tile the
  S/exp/AV work is restricted to the valid column range; only the one
  128-wide boundary block gets the additive triangular mask.

  RoPE: head dims host-permuted so each 32-partition block holds
  [re pairs | im pairs] (stream_shuffle permutes only within 32-partition
  blocks); out = psum*cos2 + swap16(psum*sin2s).

  PSUM pools are managed manually where the LIFO stack matters: psqk
  uses 4 banks with pst pre-allocated on the other 4, so the first
  attention S-pairs do not wait on psqk's release (last rope read).
  A dummy gpsimd partition_broadcast at kernel start absorbs the ~7us
  ucode library load that would otherwise stall the first normalization.
"""

import sys

if "/opt/trn_rl_repo" not in sys.path:
    sys.path.insert(0, "/opt/trn_rl_repo")

import numpy as np

import concourse.bass as bass
import concourse.tile as tile
from concourse import bacc, mybir
from concourse.bass_utils import run_bass_kernel_spmd

F32 = mybir.dt.float32
F16 = mybir.dt.float16

B, T, C = 2, 2048, 2048
NH, HD = 16, 128
NHL = 4            # heads per core
D_LOC = NHL * HD   # 512 local head dims
N_CORES = 8
SCALE = 1.0 / float(np.sqrt(HD))
NEG = -30000.0     # big enough: exp((S+NEG)*SCALE) == 0 for |S| < ~1000

CC = C // 128      # 16 contraction chunks
KC = T // 128      # 16 key chunks
QT = 512           # q tile
NQT = T // QT      # 4 q tiles

_compiled = None


def _build():
    nc = bacc.Bacc("TRN2", target_bir_lowering=False, debug=False)

    xT_d = nc.dram_tensor("xT", [C, T], F16, kind="ExternalInput")
    wq_d = nc.dram_tensor("wq", [NHL, 128, CC, 128], F16, kind="ExternalInput")
    wk_d = nc.dram_tensor("wk", [NHL, 128, CC, 128], F16, kind="ExternalInput")
    wv_d = nc.dram_tensor("wv", [128, CC, D_LOC], F16, kind="ExternalInput")
    w2_d = nc.dram_tensor("w2", [128, NHL, C], F16, kind="ExternalInput")
    cos2_d = nc.dram_tensor("cos2", [128, T], F16, kind="ExternalInput")
    sin2s_d = nc.dram_tensor("sin2s", [128, T], F16, kind="ExternalInput")
    tri_d = nc.dram_tensor("tri", [128, 128], F16, kind="ExternalInput")
    out_d = nc.dram_tensor("out", [T, C], F16, kind="ExternalOutput")

    swap_mask = list(range(16, 32)) + list(range(16))

    with tile.TileContext(nc) as tc, \
         tc.tile_pool(name="persist", bufs=1) as persist, \
         tc.tile_pool(name="px", bufs=1) as px:
        # persistent tiles
        qkTs = [persist.tile([128, T], F16, tag=f"qkT{j}", name=f"qkT{j}")
                for j in range(8)]
        v_sbs = [persist.tile([128, D_LOC], F16, tag=f"vsb{k}",
                              name=f"vsb{k}") for k in range(KC)]
        yTs = [persist.tile([128, T], F16, tag=f"yT{h}", name=f"yT{h}")
               for h in range(NHL)]
        tri_sb = persist.tile([128, 128], F16, tag="tri")
        cos2 = persist.tile([128, T], F16, tag="cos2")
        sin2s = persist.tile([128, T], F16, tag="sin2s")
        w2_sb = persist.tile([128, NHL, C], F16, tag="w2")
        ones_sb = persist.tile([128, 1], F16, tag="ones")
        gwarm = persist.tile([128, 1], F16, tag="gwarm")
        nc.vector.memset(ones_sb, 1.0)
        # touch gpsimd immediately: its partition_broadcast ucode library
        # load (~7us) then happens during the V phase instead of stalling
        # the first attention block's normalization.
        nc.gpsimd.partition_broadcast(gwarm, ones_sb[0:1, :])

        xs = px.tile([128, CC, T], F16, tag="xs")

        # ---- V phase: pass A is cc-outer so the PE starts on the first
        # x chunk; it only touches the first half of each token chunk, so
        # only 6 MB (wv + x halves) must arrive during it.  Pass B is
        # tch-outer with inline copies so the V->QK handoff is short. ----
        wv_sb = persist.tile([128, CC, D_LOC], F16, tag="wv")
        with tc.tile_pool(name="pw", bufs=4) as pw, \
             tc.tile_pool(name="rope", bufs=2) as prope:
          with tc.tile_pool(name="psv", bufs=8, space="PSUM") as psv:
            for cc in range(CC):
                nc.sync.dma_start(out=wv_sb[:, cc, :], in_=wv_d.ap()[:, cc, :])
                nsplit = 4 if cc == 0 else 2
                for s in range(nsplit):
                    w = 1024 // nsplit
                    nc.sync.dma_start(
                        out=xs[:, cc, s * w:(s + 1) * w],
                        in_=xT_d.ap()[cc * 128:(cc + 1) * 128,
                                      s * w:(s + 1) * w],
                    )
            for cc in range(CC):
                nc.sync.dma_start(
                    out=xs[:, cc, 1024:],
                    in_=xT_d.ap()[cc * 128:(cc + 1) * 128, 1024:],
                )
            nc.sync.dma_start(out=cos2, in_=cos2_d.ap())
            nc.sync.dma_start(out=sin2s, in_=sin2s_d.ap())
            nc.sync.dma_start(out=tri_sb, in_=tri_d.ap())

            def v_copy(tch, pv):
                if tch % 2 == 0:
                    nc.scalar.copy(v_sbs[tch], pv)
                else:
                    nc.vector.tensor_copy(v_sbs[tch], pv)

            # pass A: token chunks 0..7 (first half of each x chunk)
            pvs = [psv.tile([128, D_LOC], F32, tag="pv", name=f"pvA{i}")
                   for i in range(8)]
            for cc in range(CC):
                for i in range(8):
                    nc.tensor.matmul(
                        pvs[i],
                        xs[:, cc, i * 128:(i + 1) * 128],
                        wv_sb[:, cc, :],
                        start=(cc == 0), stop=(cc == CC - 1),
                    )
            for i in range(8):
                v_copy(i, pvs[i])
            # pass B: token chunks 8..11, x fully resident.  Chunks
            # 12..15 are deferred into the attention phase as PE filler
            # for the qt0 blocks (they are only read by qt3 blocks).
            for tch in range(8, 12):
                pv = psv.tile([128, D_LOC], F32, tag="pv", name=f"pvB{tch}")
                for cc in range(CC):
                    nc.tensor.matmul(
                        pv,
                        xs[:, cc, tch * 128:(tch + 1) * 128],
                        wv_sb[:, cc, :],
                        start=(cc == 0), stop=(cc == CC - 1),
                    )
                v_copy(tch, pv)

          # ---- QK phase: Q^T,K^T in [d, t] layout + RoPE ----
          # psqk uses only 4 banks so the attention pst pool can live on
          # the other 4 while QK runs: the first S-pairs then do not wait
          # on psqk's release (which is gated by the last rope read).
          # PSUM pool stack is LIFO: pst first, psqk on top.
          pst = tc.alloc_tile_pool(name="pst", bufs=2, space="PSUM")
          psqk = tc.alloc_tile_pool(name="psqk", bufs=4, space="PSUM")
          if True:
            # K heads first: the attention S-matmuls need the rope'd K
            # columns as stationary operands right after QK ends.
            for jc in (4, 5, 6, 7, 0, 1, 2, 3):
                w_src = (wq_d if jc < 4 else wk_d).ap()[jc % 4]
                w_sb = pw.tile([128, CC, 128], F16, tag="w",
                               name=f"w_sb{jc}")
                nc.sync.dma_start(out=w_sb, in_=w_src)
                for tt in range(NQT):
                    gt0 = tt * QT
                    ps = psqk.tile([128, QT], F32, tag="qk",
                                   name=f"psqk{jc}_{tt}")
                    for cc in range(CC):
                        nc.tensor.matmul(
                            ps, w_sb[:, cc, :],
                            xs[:, cc, gt0:gt0 + QT],
                            start=(cc == 0), stop=(cc == CC - 1),
                        )
                    u = prope.tile([128, QT], F16, tag="u", name=f"u{jc}{tt}")
                    v = prope.tile([128, QT], F16, tag="v", name=f"v{jc}{tt}")
                    w = prope.tile([128, QT], F16, tag="w", name=f"w{jc}{tt}")
                    nc.vector.tensor_mul(u, ps, cos2[:, gt0:gt0 + QT])
                    nc.vector.tensor_mul(v, ps, sin2s[:, gt0:gt0 + QT])
                    nc.vector.stream_shuffle(w, v, swap_mask)
                    nc.vector.tensor_add(qkTs[jc][:, gt0:gt0 + QT], u, w)

        # ---- ATT + PROJ, q-tile outer ----
        # S chunks paired into [128, 1024] PSUM tiles (one exp per pair);
        # softmax denominator accumulated on DVE (asum) + one ones-matmul
        # per (h, qt) block whose [1, QT] output shares the proj PSUM tag.
        psqk.release()
        psy = tc.alloc_tile_pool(name="psy", bufs=2, space="PSUM")
        pso = tc.alloc_tile_pool(name="pso", bufs=2, space="PSUM")
        with tc.tile_pool(name="att", bufs=3) as patt, \
             tc.tile_pool(name="acc", bufs=2) as pacc, \
             tc.tile_pool(name="nrm", bufs=2) as pnrm, \
             tc.tile_pool(name="outp", bufs=2) as pout:

            nc.sync.dma_start(out=w2_sb, in_=w2_d.ap())

            class Blk:
                """One (head, q-tile) attention block."""

                def __init__(self, h, qt):
                    self.h, self.qt = h, qt
                    self.q0 = qt * QT
                    self.nkc = 4 * qt + 4  # valid k chunks (causal)
                    self.np = self.nkc // 2
                    self.a_pairs = [None] * self.np

                def lo(self, kc):
                    return max(0, (kc - 4 * self.qt) * 128)

                def emit_pair(self, p):
                    h, qt = self.h, self.qt
                    sps = pst.tile([128, 2 * QT], F32, tag="st",
                                   name=f"sps{h}{qt}{p}")
                    for kc in (2 * p, 2 * p + 1):
                        off = (kc % 2) * QT
                        l = self.lo(kc)
                        nc.tensor.matmul(
                            sps[:, off + l:off + QT],
                            qkTs[4 + h][:, kc * 128:(kc + 1) * 128],
                            qkTs[h][:, self.q0 + l:self.q0 + QT],
                            start=True, stop=True,
                        )
                        if kc - 4 * qt >= 0:
                            nc.vector.tensor_add(
                                sps[:, off + l:off + l + 128],
                                sps[:, off + l:off + l + 128], tri_sb)
                    a = patt.tile([128, 2 * QT], F16, tag="a",
                                  name=f"a{h}{qt}{p}")
                    l0 = self.lo(2 * p)
                    nc.scalar.activation(
                        a[:, l0:], sps[:, l0:],
                        mybir.ActivationFunctionType.Exp, scale=SCALE,
                    )
                    self.a_pairs[p] = a

            # block emission order, with proj(qt) after the rotated
            # att(0, qt+1) block
            order = [(0, 0)]
            proj_after = {}
            for qt in range(NQT):
                for h in range(1, NHL):
                    order.append((h, qt))
                if qt + 1 < NQT:
                    order.append((0, qt + 1))
                proj_after[len(order) - 1] = qt
            blks = [Blk(h, qt) for h, qt in order]

            # global S-pair stream: keep LA pairs emitted ahead of the
            # consume cursor so the PE always has exp'd tiles ready,
            # including across block seams.
            pair_seq = [(bi, p) for bi, b in enumerate(blks)
                        for p in range(b.np)]
            emitted = [0]

            def emit_upto(n):
                while emitted[0] < min(n, len(pair_seq)):
                    bi, p = pair_seq[emitted[0]]
                    blks[bi].emit_pair(p)
                    emitted[0] += 1

            # proj work is queued as (qc, ct) groups and drained one group
            # per consumed S-pair inside the NEXT q-tile's attention
            # blocks: ScalarE then never idles during proj, its exp stream
            # keeps pacing the S/AV pipeline.
            proj_groups = []
            osb_tiles = {}

            def emit_proj_group(qc, ct):
                if ct == 0:
                    osb_tiles[qc] = pout.tile([128, C], F16, tag="o",
                                              name=f"osb{qc}")
                osb = osb_tiles[qc]
                ops = pso.tile([128, QT], F32, tag="op", name=f"ops{qc}{ct}")
                for hh in range(NHL):
                    nc.tensor.matmul(
                        ops,
                        yTs[hh][:, qc * 128:(qc + 1) * 128],
                        w2_sb[:, hh, ct * QT:(ct + 1) * QT],
                        start=(hh == 0), stop=(hh == NHL - 1),
                    )
                if ct % 2 == 0:
                    nc.scalar.copy(osb[:, ct * QT:(ct + 1) * QT], ops)
                else:
                    nc.vector.tensor_copy(osb[:, ct * QT:(ct + 1) * QT], ops)
                if qc == T // 128 - 1:
                    # split the final output DMA to shrink the tail
                    nc.sync.dma_start(
                        out=out_d.ap()[qc * 128:(qc + 1) * 128,
                                       ct * QT:(ct + 1) * QT],
                        in_=osb[:, ct * QT:(ct + 1) * QT],
                    )
                elif ct == C // QT - 1:
                    nc.sync.dma_start(
                        out=out_d.ap()[qc * 128:(qc + 1) * 128, :],
                        in_=osb,
                    )

            # deferred V token-chunks (12..15), emitted as PE filler
            # during the qt0 attention blocks which have no proj work yet
            v_fillers = list(range(12, KC))

            def emit_v_filler(tch):
                pv = pso.tile([128, D_LOC], F32, tag="op", name=f"pvD{tch}")
                for cc in range(CC):
                    nc.tensor.matmul(
                        pv,
                        xs[:, cc, tch * 128:(tch + 1) * 128],
                        wv_sb[:, cc, :],
                        start=(cc == 0), stop=(cc == CC - 1),
                    )
                nc.scalar.copy(v_sbs[tch], pv)

            def drain_proj(n):
                for _ in range(n):
                    if v_fillers:
                        emit_v_filler(v_fillers.pop(0))
                    elif proj_groups:
                        qc, ct = proj_groups.pop(0)
                        emit_proj_group(qc, ct)
                    else:
                        return

            def push_proj(qt):
                for qc in range(4 * qt, 4 * qt + 4):
                    for ct in range(C // QT):
                        proj_groups.append((qc, ct))

            LA = 2
            gpair = [0]
            # the yT-normalize multiply of a block is deferred into the
            # NEXT block's consume loop: it waits on the gpsimd broadcast
            # and would otherwise head-of-line-block the in-order DVE
            # queue right when the next block's asum adds need to run.
            pending_mul = []

            def flush_muls():
                while pending_mul:
                    yps_, rb_, h_, q0_ = pending_mul.pop(0)
                    nc.vector.tensor_mul(yTs[h_][:, q0_:q0_ + QT], yps_, rb_)

            for bi, b in enumerate(blks):
                h, qt, q0, nkc = b.h, b.qt, b.q0, b.nkc
                yps = psy.tile([128, QT], F32, tag="y", name=f"yps{h}{qt}")
                asum = pacc.tile([128, QT], F16, tag="as", name=f"as{h}{qt}")
                for p in range(b.np):
                    emit_upto(gpair[0] + 1 + LA)
                    a = b.a_pairs[p]
                    for kc in (2 * p, 2 * p + 1):
                        off = (kc % 2) * QT
                        l = b.lo(kc)
                        nc.tensor.matmul(
                            yps[:, l:], v_sbs[kc][:, h * HD:(h + 1) * HD],
                            a[:, off + l:off + QT],
                            start=(kc == 0), stop=(kc == nkc - 1),
                        )
                        if kc == 0:
                            nc.vector.tensor_copy(asum, a[:, 0:QT])
                        else:
                            nc.vector.tensor_add(
                                asum[:, l:], asum[:, l:],
                                a[:, off + l:off + QT])
                    gpair[0] += 1
                    if p == 0:
                        flush_muls()
                    else:
                        drain_proj(1)
                dps = pso.tile([1, QT], F32, tag="op", name=f"dps{h}{qt}")
                nc.tensor.matmul(dps, ones_sb, asum, start=True, stop=True)
                rinv = pnrm.tile([1, QT], F32, tag="rinv", name=f"ri{h}{qt}")
                nc.vector.reciprocal_approx_fast(rinv, dps)
                rb = pnrm.tile([128, QT], F32, tag="rb", name=f"rb{h}{qt}")
                nc.gpsimd.partition_broadcast(rb, rinv)
                pending_mul.append((yps, rb, h, q0))
                if bi in proj_after:
                    pqt = proj_after[bi]
                    drain_proj(64)  # flush fillers + previous qt's proj
                    if pqt == NQT - 1:
                        # final q-tile: its own (3,3) multiply is pending
                        flush_muls()
                        push_proj(pqt)
                        drain_proj(64)
                    else:
                        push_proj(pqt)
            flush_muls()
            drain_proj(64)
        pso.release()
        psy.release()
        pst.release()

    nc.compile()
    return nc


def _prep_core_inputs(core, x16, W_attn, W_proj, cos2, sin2s, tri):
    b = core // 4
    g = core % 4
    heads = [g * NHL + i for i in range(NHL)]
    # stream_shuffle permutes within 32-partition blocks only: lay out each
    # block as [re pairs 16b..16b+15 | im pairs 16b..16b+15]
    perm = np.concatenate(
        [np.r_[2 * (16 * blk + np.arange(16)),
               2 * (16 * blk + np.arange(16)) + 1]
         for blk in range(4)]
    )

    xT = np.ascontiguousarray(x16[b].T)

    def qk_blocks(base):
        blocks = []
        for h in heads:
            blk = W_attn[:, base + h * HD: base + (h + 1) * HD][:, perm]
            blocks.append(blk.reshape(CC, 128, HD).transpose(1, 0, 2))
        return np.ascontiguousarray(np.stack(blocks, axis=0)).astype(np.float16)

    wq = qk_blocks(0)
    wk = qk_blocks(C)
    wv = np.concatenate(
        [W_attn[:, 2 * C + h * HD: 2 * C + (h + 1) * HD] for h in heads],
        axis=1,
    )  # (C, D_LOC)
    wv = np.ascontiguousarray(
        wv.reshape(CC, 128, D_LOC).transpose(1, 0, 2)).astype(np.float16)
    w2 = np.ascontiguousarray(
        np.stack([W_proj[h * HD:(h + 1) * HD, :] for h in heads], axis=0)
        .transpose(1, 0, 2)
    ).astype(np.float16)
    return {
        "xT": xT, "wq": wq, "wk": wk, "wv": wv, "w2": w2,
        "cos2": cos2, "sin2s": sin2s, "tri": tri,
    }


def _run(inputs, trace=False):
    global _compiled
    x = np.asarray(inputs["x"], dtype=np.float32)
    W_attn = np.asarray(inputs["W_attn"], dtype=np.float32)
    W_proj = np.asarray(inputs["W_proj"], dtype=np.float32)
    fc = np.asarray(inputs["freqs_cos"], dtype=np.float32)
    fs = np.asarray(inputs["freqs_sin"], dtype=np.float32)

    x16 = x.astype(np.float16)

    cosT = np.ascontiguousarray(fc.T)            # (64, T)
    sinT = np.ascontiguousarray(fs.T)
    # per 32-partition block b: partitions [0:16] carry cos/sin of pairs
    # 16b..16b+15 (re half, +sin), [16:32] the same freqs (im half, -sin)
    cos2 = np.concatenate(
        [np.concatenate([cosT[16 * blk:16 * (blk + 1)]] * 2, axis=0)
         for blk in range(4)], axis=0)           # (128, T)
    sin2s = np.concatenate(
        [np.concatenate([sinT[16 * blk:16 * (blk + 1)],
                         -sinT[16 * blk:16 * (blk + 1)]], axis=0)
         for blk in range(4)], axis=0)
    cos2 = np.ascontiguousarray(cos2).astype(np.float16)
    sin2s = np.ascontiguousarray(sin2s).astype(np.float16)

    # triangular boundary-block mask: row k (local), col c (local):
    # valid (0) when k <= c else NEG
    ki = np.arange(128)[:, None]
    ci = np.arange(128)[None, :]
    tri = np.ascontiguousarray(
        np.where(ki <= ci, 0.0, NEG).astype(np.float16))  # (128, 128)

    if _compiled is None:
        _compiled = _build()
    nc = _compiled

    in_maps = [
        _prep_core_inputs(c, x16, W_attn, W_proj, cos2, sin2s, tri)
        for c in range(N_CORES)
    ]
    res = run_bass_kernel_spmd(
        nc, in_maps, core_ids=list(range(N_CORES)), trace=trace)

    out = np.zeros((B, T, C), dtype=np.float32)
    for c in range(N_CORES):
        out[c // 4] += res.results[c]["out"]
    return out, res


def kernel(**inputs) -> np.ndarray:
    out, _ = _run(inputs, trace=False)
    return out
